# revision 11
# baseline (speedup 1.0000x reference)
"""
DenseEquivariantIrrep kernel for 8x Trainium2 NeuronCores.

Math: the reference computes, per batch row b:
    y[b, f, s] = sum_{c,t} x[b, c, t] * W[(c,t), (f,s)] + bias[f]
where W folds the group-Fourier transform (fwd), the per-irrep block
matmul with the kernel, and the inverse transform (inv).  W depends only
on (kernel, irreps) which are tiny, so it is folded on the host; the
device does the batch-scaled work: a [1536, 1536]^T x [1536, 4096]
matmul per core (8-way batch-parallel, no cross-core communication).

Layout strategy: the host marshals x into x^T (feature-major) fp16 and
un-marshals y^T afterward, so the device runs a pure LDWEIGHTS+MATMUL
stream with zero PE transposes:
    y^T[fs, b] = sum_j W[j, fs]^T @ x^T[j, b]   (12 accumulating steps)
with the bias-add fused into the PSUM->SBUF copyback (DVE/ACT
alternating) and fp16 DMA in/out.
"""

import numpy as np

import concourse.bass as bass
import concourse.mybir as mybir
from concourse import bacc
from concourse.tile import TileContext
from concourse.bass_utils import run_bass_kernel_spmd

N_CORES = 8
B, C, F, NS = 32768, 32, 32, 48
CT = C * NS   # 1536 contraction size
FS = F * NS   # 1536 output features
BS = B // N_CORES  # 4096 batch columns per core
BB = 512           # b-columns per block (PSUM bank width in fp32)
KT = CT // 128     # 12 K tiles
MT = FS // 128     # 12 M (output) tiles

IO_DT = mybir.dt.float16   # wire dtype for x^T / W / y^T


def _host_fold(kernel, bias, irreps_d1, irreps_d2, irreps_d3):
    """Fold fwd/inv Fourier matrices and kernel into W[(c,t),(f,s)] + bias[(f,s)]."""
    groups = [np.asarray(irreps_d1), np.asarray(irreps_d2), np.asarray(irreps_d3)]
    n = NS
    fwd = np.concatenate(
        [g.transpose(1, 0, 2, 3).reshape(n, -1) for g in groups], axis=1
    ).astype(np.float64)
    inv = np.concatenate(
        [g.transpose(1, 0, 2, 3).reshape(n, -1) * (g.shape[-1] / n) for g in groups],
        axis=1,
    ).T.astype(np.float64)
    kh = np.asarray(kernel).astype(np.float64) @ fwd  # [F, C, 48]
    W = np.zeros((C, NS, F, NS), np.float64)
    off = 0
    for g in groups:
        ni, d = g.shape[0], g.shape[-1]
        for _ in range(ni):
            fw_n = fwd[:, off : off + d * d].reshape(n, d, d)         # [t, p, r]
            kh_n = kh[:, :, off : off + d * d].reshape(F, C, d, d)    # [f, c, r, q]
            iv_n = inv[off : off + d * d, :].reshape(d, d, n)         # [p, q, s]
            W += np.einsum("tpr,fcrq,pqs->ctfs", fw_n, kh_n, iv_n, optimize=True)
            off += d * d
    Wflat = np.ascontiguousarray(W.reshape(CT, FS)).astype(np.float32)
    bias_fs = np.repeat(np.asarray(bias).astype(np.float32), NS)  # [FS], f-major
    # partition-major layout for the device: bias_pm[p, m] = bias_fs[m*128+p]
    bias_pm = np.ascontiguousarray(bias_fs.reshape(MT, 128).T)
    return Wflat, bias_pm


def _w_tiles(Wflat):
    """[CT, FS] -> [KT*MT, 128, 128] fp16 tiles, index t = j*MT + m."""
    wt = Wflat.reshape(KT, 128, MT, 128).transpose(0, 2, 1, 3)
    return np.ascontiguousarray(wt.reshape(KT * MT, 128, 128)).astype(np.float16)


def build_kernel(nc: bass.Bass, bs: int = BS, reps: int = 1, pair: bool = False):
    """Emit the per-core kernel into `nc`. bs = batch columns for this build.

    reps > 1 wraps the whole pipeline in a hardware loop (for timing)."""
    assert bs % (2 * BB if pair else BB) == 0
    import contextlib

    xt_d = nc.dram_tensor("xt", [KT, 128, bs], IO_DT, kind="ExternalInput")
    w_d = nc.dram_tensor("wt", [KT * MT, 128, 128], IO_DT, kind="ExternalInput")
    b_d = nc.dram_tensor("bias_pm", [128, MT], mybir.dt.float32, kind="ExternalInput")
    yt_d = nc.dram_tensor("yt", [MT, 128, bs], IO_DT, kind="ExternalOutput")

    nbb = bs // BB

    with TileContext(nc) as tc:
        with (
            tc.tile_pool(name="singles", bufs=1) as singles,
            tc.tile_pool(name="xin", bufs=3) as xin_pool,
            tc.tile_pool(name="yout", bufs=3) as yout_pool,
            tc.tile_pool(name="py", bufs=6, space="PSUM") as py_pool,
        ):
            w_sb = singles.tile([128, KT * MT, 128], IO_DT)
            for t in range(KT * MT):
                nc.sync.dma_start(out=w_sb[:, t, :], in_=w_d[t, :, :])
            bias_sb = singles.tile([128, MT], mybir.dt.float32)
            nc.sync.dma_start(out=bias_sb, in_=b_d[:, :])

            # process pairs of 512-col blocks: one stationary load feeds two
            # matmuls (into two PSUM banks), halving weight-load overhead
            group = 2 * BB if pair else BB
            ngrp = bs // group
            rep_ctx = (
                tc.For_i(0, reps, 1, hint_engines=(mybir.EngineType.PE,))
                if reps > 1
                else contextlib.nullcontext()
            )
            with rep_ctx:
                for bp in range(ngrp):
                    c0 = bp * group
                    yo_blk = yout_pool.tile([128, MT, group], IO_DT, tag="yo")
                    xt = xin_pool.tile([128, KT, group], IO_DT, tag="xin")
                    nc.sync.dma_start(
                        out=xt,
                        in_=xt_d[:, :, c0 : c0 + group].rearrange("j p c -> p j c"),
                    )
                    for m in range(MT):
                        pys = [
                            py_pool.tile(
                                [128, BB], mybir.dt.float32, tag="py",
                                name=f"py_{bp}_{m}_{h}",
                            )
                            for h in range(group // BB)
                        ]
                        for j in range(KT):
                            w = w_sb[:, j * MT + m, :]
                            for h, py in enumerate(pys):
                                nc.tensor.matmul(
                                    py, w, xt[:, j, h * BB : (h + 1) * BB],
                                    start=(j == 0), stop=(j == KT - 1),
                                )
                        for h, py in enumerate(pys):
                            # bias-add + fp32->fp16 cast on copyback, DVE/ACT split
                            if (h + m) % 2 == 0:
                                nc.vector.tensor_scalar_add(
                                    yo_blk[:, m, h * BB : (h + 1) * BB],
                                    py, bias_sb[:, m : m + 1],
                                )
                            else:
                                nc.scalar.activation(
                                    yo_blk[:, m, h * BB : (h + 1) * BB], py,
                                    mybir.ActivationFunctionType.Identity,
                                    bias=bias_sb[:, m : m + 1],
                                )
                    nc.sync.dma_start(
                        out=yt_d[:, :, c0 : c0 + group].rearrange("m p c -> p m c"),
                        in_=yo_blk,
                    )
    return nc


def _marshal(x):
    """x [B, C, NS] f32 -> per-core x^T [CT, BS] fp16 (contiguous)."""
    xf = np.asarray(x, np.float16).reshape(N_CORES, BS, CT)
    return np.ascontiguousarray(xf.transpose(0, 2, 1))  # [N_CORES, CT, BS]


def _run(x, Wflat, bias_pm, trace=False, tmpdir=None):
    nc = bacc.Bacc("TRN2", target_bir_lowering=False)
    build_kernel(nc, BS)
    nc.compile()
    xt = _marshal(x)
    wt = _w_tiles(Wflat)
    in_maps = [
        {"xt": xt[i].reshape(KT, 128, BS), "wt": wt, "bias_pm": bias_pm}
        for i in range(N_CORES)
    ]
    res = run_bass_kernel_spmd(
        nc, in_maps, list(range(N_CORES)), trace=trace, tmpdir=tmpdir
    )
    yt = np.stack(
        [res.results[i]["yt"].reshape(FS, BS) for i in range(N_CORES)]
    )  # [8, FS, BS]
    y = yt.transpose(0, 2, 1).astype(np.float32).reshape(B, F, NS)
    return y, res


def kernel(x, kernel, bias, irreps_d1, irreps_d2, irreps_d3):
    Wflat, bias_pm = _host_fold(kernel, bias, irreps_d1, irreps_d2, irreps_d3)
    y, _ = _run(np.asarray(x, dtype=np.float32), Wflat, bias_pm)
    return y


# revision 13
# speedup vs baseline: 1.0935x; 1.0935x over previous
"""
DenseEquivariantIrrep kernel for 8x Trainium2 NeuronCores.

Math: the reference computes, per batch row b:
    y[b, f, s] = sum_{c,t} x[b, c, t] * W[(c,t), (f,s)] + bias[f]
where W folds the group-Fourier transform (fwd), the per-irrep block
matmul with the kernel, and the inverse transform (inv).  W depends only
on (kernel, irreps) which are tiny, so it is folded on the host; the
device does the batch-scaled work: a [1536, 1536]^T x [1536, 4096]
matmul per core (8-way batch-parallel, no cross-core communication).

Layout strategy: the host marshals x into x^T (feature-major) fp16 and
un-marshals y^T afterward, so the device runs a pure LDWEIGHTS+MATMUL
stream with zero PE transposes:
    y^T[fs, b] = sum_j W[j, fs]^T @ x^T[j, b]   (12 accumulating steps)
with the bias-add fused into the PSUM->SBUF copyback (DVE/ACT
alternating) and fp16 DMA in/out.
"""

import numpy as np

import concourse.bass as bass
import concourse.mybir as mybir
from concourse import bacc
from concourse.tile import TileContext
from concourse.bass_utils import run_bass_kernel_spmd

N_CORES = 8
B, C, F, NS = 32768, 32, 32, 48
CT = C * NS   # 1536 contraction size
FS = F * NS   # 1536 output features
BS = B // N_CORES  # 4096 batch columns per core
BB = 512           # b-columns per block (PSUM bank width in fp32)
KT = CT // 128     # 12 K tiles
MT = FS // 128     # 12 M (output) tiles

IO_DT = mybir.dt.float16   # wire dtype for x^T / W / y^T


def _host_fold(kernel, bias, irreps_d1, irreps_d2, irreps_d3):
    """Fold fwd/inv Fourier matrices and kernel into W[(c,t),(f,s)] + bias[(f,s)]."""
    groups = [np.asarray(irreps_d1), np.asarray(irreps_d2), np.asarray(irreps_d3)]
    n = NS
    fwd = np.concatenate(
        [g.transpose(1, 0, 2, 3).reshape(n, -1) for g in groups], axis=1
    ).astype(np.float64)
    inv = np.concatenate(
        [g.transpose(1, 0, 2, 3).reshape(n, -1) * (g.shape[-1] / n) for g in groups],
        axis=1,
    ).T.astype(np.float64)
    kh = np.asarray(kernel).astype(np.float64) @ fwd  # [F, C, 48]
    W = np.zeros((C, NS, F, NS), np.float64)
    off = 0
    for g in groups:
        ni, d = g.shape[0], g.shape[-1]
        for _ in range(ni):
            fw_n = fwd[:, off : off + d * d].reshape(n, d, d)         # [t, p, r]
            kh_n = kh[:, :, off : off + d * d].reshape(F, C, d, d)    # [f, c, r, q]
            iv_n = inv[off : off + d * d, :].reshape(d, d, n)         # [p, q, s]
            W += np.einsum("tpr,fcrq,pqs->ctfs", fw_n, kh_n, iv_n, optimize=True)
            off += d * d
    Wflat = np.ascontiguousarray(W.reshape(CT, FS)).astype(np.float32)
    bias_fs = np.repeat(np.asarray(bias).astype(np.float32), NS)  # [FS], f-major
    # partition-major layout for the device: bias_pm[p, m] = bias_fs[m*128+p]
    bias_pm = np.ascontiguousarray(bias_fs.reshape(MT, 128).T)
    return Wflat, bias_pm


def _w_tiles(Wflat):
    """[CT, FS] -> [KT*MT, 128, 128] fp16 tiles, index t = j*MT + m."""
    wt = Wflat.reshape(KT, 128, MT, 128).transpose(0, 2, 1, 3)
    return np.ascontiguousarray(wt.reshape(KT * MT, 128, 128)).astype(np.float16)


def build_kernel(nc: bass.Bass, bs: int = BS, reps: int = 1, pair: bool = False,
                 fixed_w: bool = False, out_ring: str = "sync"):
    """Emit the per-core kernel into `nc`. bs = batch columns for this build.

    reps > 1 wraps the whole pipeline in a hardware loop (for timing)."""
    assert bs % (2 * BB if pair else BB) == 0
    import contextlib

    xt_d = nc.dram_tensor("xt", [KT, 128, bs], IO_DT, kind="ExternalInput")
    w_d = nc.dram_tensor("wt", [KT * MT, 128, 128], IO_DT, kind="ExternalInput")
    b_d = nc.dram_tensor("bias_pm", [128, MT], mybir.dt.float32, kind="ExternalInput")
    yt_d = nc.dram_tensor("yt", [MT, 128, bs], IO_DT, kind="ExternalOutput")

    nbb = bs // BB

    with TileContext(nc) as tc:
        with (
            tc.tile_pool(name="singles", bufs=1) as singles,
            tc.tile_pool(name="xin", bufs=3) as xin_pool,
            tc.tile_pool(name="yout", bufs=6) as yout_pool,
            tc.tile_pool(name="py", bufs=6, space="PSUM") as py_pool,
        ):
            w_sb = singles.tile([128, KT * MT, 128], IO_DT)
            for t in range(KT * MT):
                nc.sync.dma_start(out=w_sb[:, t, :], in_=w_d[t, :, :])
            bias_sb = singles.tile([128, MT], mybir.dt.float32)
            nc.sync.dma_start(out=bias_sb, in_=b_d[:, :])

            # process pairs of 512-col blocks: one stationary load feeds two
            # matmuls (into two PSUM banks), halving weight-load overhead
            group = 2 * BB if pair else BB
            ngrp = bs // group
            rep_ctx = (
                tc.For_i(0, reps, 1, hint_engines=(mybir.EngineType.PE,))
                if reps > 1
                else contextlib.nullcontext()
            )
            with rep_ctx:
                for bp in range(ngrp):
                    c0 = bp * group
                    xt = xin_pool.tile([128, KT, group], IO_DT, tag="xin")
                    for j in range(KT):
                        nc.sync.dma_start(
                            out=xt[:, j, :], in_=xt_d[j, :, c0 : c0 + group]
                        )
                    for m in range(MT):
                        pys = [
                            py_pool.tile(
                                [128, BB], mybir.dt.float32, tag="py",
                                name=f"py_{bp}_{m}_{h}",
                            )
                            for h in range(group // BB)
                        ]
                        for j in range(KT):
                            w = w_sb[:, 0 if fixed_w else j * MT + m, :]
                            for h, py in enumerate(pys):
                                nc.tensor.matmul(
                                    py, w, xt[:, j, h * BB : (h + 1) * BB],
                                    start=(j == 0), stop=(j == KT - 1),
                                )
                        for h, py in enumerate(pys):
                            yo = yout_pool.tile([128, BB], IO_DT, tag="yo")
                            # bias-add + fp32->fp16 cast on copyback, DVE/ACT split
                            if (h + m) % 2 == 0:
                                nc.vector.tensor_scalar_add(
                                    yo, py, bias_sb[:, m : m + 1]
                                )
                            else:
                                nc.scalar.activation(
                                    yo, py,
                                    mybir.ActivationFunctionType.Identity,
                                    bias=bias_sb[:, m : m + 1],
                                )
                            out_eng = (
                                nc.scalar if out_ring == "act" else nc.sync
                            )
                            out_eng.dma_start(
                                out=yt_d[m, :, c0 + h * BB : c0 + (h + 1) * BB],
                                in_=yo,
                            )
    return nc


def _marshal(x):
    """x [B, C, NS] f32 -> per-core x^T [CT, BS] fp16 (contiguous)."""
    xf = np.asarray(x, np.float16).reshape(N_CORES, BS, CT)
    return np.ascontiguousarray(xf.transpose(0, 2, 1))  # [N_CORES, CT, BS]


def _run(x, Wflat, bias_pm, trace=False, tmpdir=None):
    nc = bacc.Bacc("TRN2", target_bir_lowering=False)
    build_kernel(nc, BS)
    nc.compile()
    xt = _marshal(x)
    wt = _w_tiles(Wflat)
    in_maps = [
        {"xt": xt[i].reshape(KT, 128, BS), "wt": wt, "bias_pm": bias_pm}
        for i in range(N_CORES)
    ]
    res = run_bass_kernel_spmd(
        nc, in_maps, list(range(N_CORES)), trace=trace, tmpdir=tmpdir
    )
    yt = np.stack(
        [res.results[i]["yt"].reshape(FS, BS) for i in range(N_CORES)]
    )  # [8, FS, BS]
    y = yt.transpose(0, 2, 1).astype(np.float32).reshape(B, F, NS)
    return y, res


def kernel(x, kernel, bias, irreps_d1, irreps_d2, irreps_d3):
    Wflat, bias_pm = _host_fold(kernel, bias, irreps_d1, irreps_d2, irreps_d3)
    y, _ = _run(np.asarray(x, dtype=np.float32), Wflat, bias_pm)
    return y


# revision 14
# speedup vs baseline: 1.9574x; 1.7901x over previous
"""
DenseEquivariantIrrep kernel for 8x Trainium2 NeuronCores.

Math: the reference computes, per batch row b:
    y[b, f, s] = sum_{c,t} x[b, c, t] * W[(c,t), (f,s)] + bias[f]
where W folds the group-Fourier transform (fwd), the per-irrep block
matmul with the kernel, and the inverse transform (inv).  W depends only
on (kernel, irreps) which are tiny, so it is folded on the host; the
device does the batch-scaled work: a [1536, 1536]^T x [1536, 4096]
matmul per core (8-way batch-parallel, no cross-core communication).

Layout strategy: the host marshals x into x^T (feature-major) fp16 and
un-marshals y^T afterward, so the device runs a pure LDWEIGHTS+MATMUL
stream with zero PE transposes:
    y^T[fs, b] = sum_j W[j, fs]^T @ x^T[j, b]   (12 accumulating steps)
with the bias-add fused into the PSUM->SBUF copyback (DVE/ACT
alternating) and fp16 DMA in/out.
"""

import numpy as np

import concourse.bass as bass
import concourse.mybir as mybir
from concourse import bacc
from concourse.tile import TileContext
from concourse.bass_utils import run_bass_kernel_spmd

N_CORES = 8
B, C, F, NS = 32768, 32, 32, 48
CT = C * NS   # 1536 contraction size
FS = F * NS   # 1536 output features
BS = B // N_CORES  # 4096 batch columns per core
BB = 512           # b-columns per block (PSUM bank width in fp32)
KT = CT // 128     # 12 K tiles
MT = FS // 128     # 12 M (output) tiles

IO_DT = mybir.dt.float16   # wire dtype for x^T / W / y^T


def _host_fold(kernel, bias, irreps_d1, irreps_d2, irreps_d3):
    """Fold fwd/inv Fourier matrices and kernel into W[(c,t),(f,s)] + bias[(f,s)]."""
    groups = [np.asarray(irreps_d1), np.asarray(irreps_d2), np.asarray(irreps_d3)]
    n = NS
    fwd = np.concatenate(
        [g.transpose(1, 0, 2, 3).reshape(n, -1) for g in groups], axis=1
    ).astype(np.float64)
    inv = np.concatenate(
        [g.transpose(1, 0, 2, 3).reshape(n, -1) * (g.shape[-1] / n) for g in groups],
        axis=1,
    ).T.astype(np.float64)
    kh = np.asarray(kernel).astype(np.float64) @ fwd  # [F, C, 48]
    W = np.zeros((C, NS, F, NS), np.float64)
    off = 0
    for g in groups:
        ni, d = g.shape[0], g.shape[-1]
        for _ in range(ni):
            fw_n = fwd[:, off : off + d * d].reshape(n, d, d)         # [t, p, r]
            kh_n = kh[:, :, off : off + d * d].reshape(F, C, d, d)    # [f, c, r, q]
            iv_n = inv[off : off + d * d, :].reshape(d, d, n)         # [p, q, s]
            W += np.einsum("tpr,fcrq,pqs->ctfs", fw_n, kh_n, iv_n, optimize=True)
            off += d * d
    Wflat = np.ascontiguousarray(W.reshape(CT, FS)).astype(np.float32)
    bias_fs = np.repeat(np.asarray(bias).astype(np.float32), NS)  # [FS], f-major
    # partition-major layout for the device: bias_pm[p, m] = bias_fs[m*128+p]
    bias_pm = np.ascontiguousarray(bias_fs.reshape(MT, 128).T)
    return Wflat, bias_pm


def _w_tiles(Wflat):
    """[CT, FS] -> [MT*KT, 128, 128] fp16 tiles, index t = m*KT + j (m-major
    so the first output group's weights arrive first)."""
    wt = Wflat.reshape(KT, 128, MT, 128).transpose(2, 0, 1, 3)
    return np.ascontiguousarray(wt.reshape(MT * KT, 128, 128)).astype(np.float16)


def build_kernel(nc: bass.Bass, bs: int = BS, reps: int = 1, pair: bool = False,
                 fixed_w: bool = False, out_ring: str = "sync"):
    """Emit the per-core kernel into `nc`. bs = batch columns for this build.

    reps > 1 wraps the whole pipeline in a hardware loop (for timing)."""
    assert bs % (2 * BB if pair else BB) == 0
    import contextlib

    xt_d = nc.dram_tensor("xt", [KT, 128, bs], IO_DT, kind="ExternalInput")
    w_d = nc.dram_tensor("wt", [MT * KT, 128, 128], IO_DT, kind="ExternalInput")
    b_d = nc.dram_tensor("bias_pm", [128, MT], mybir.dt.float32, kind="ExternalInput")
    yt_d = nc.dram_tensor("yt", [MT, 128, bs], IO_DT, kind="ExternalOutput")

    nbb = bs // BB

    with TileContext(nc) as tc:
        group = 2 * BB if pair else BB
        ngrp = bs // group
        with (
            tc.tile_pool(name="singles", bufs=1) as singles,
            tc.tile_pool(name="xin", bufs=3) as xin_pool,
            tc.tile_pool(name="yout", bufs=6) as yout_pool,
            tc.tile_pool(name="py", bufs=6, space="PSUM") as py_pool,
        ):
            # prefetch the first x block ahead of the weights so the first
            # matmul group can start as soon as its 12 W tiles land
            xt0 = xin_pool.tile([128, KT, group], IO_DT, tag="xin")
            for j in range(KT):
                nc.sync.dma_start(out=xt0[:, j, :], in_=xt_d[j, :, 0:group])
            w_sb = singles.tile([128, MT * KT, 128], IO_DT)
            for t in range(MT * KT):
                nc.sync.dma_start(out=w_sb[:, t, :], in_=w_d[t, :, :])
            bias_sb = singles.tile([128, MT], mybir.dt.float32)
            nc.sync.dma_start(out=bias_sb, in_=b_d[:, :])


            rep_ctx = (
                tc.For_i(0, reps, 1, hint_engines=(mybir.EngineType.PE,))
                if reps > 1
                else contextlib.nullcontext()
            )
            with rep_ctx:
                for bp in range(ngrp):
                    c0 = bp * group
                    if bp == 0 and reps == 1:
                        xt = xt0
                    else:
                        xt = xin_pool.tile([128, KT, group], IO_DT, tag="xin")
                        for j in range(KT):
                            nc.sync.dma_start(
                                out=xt[:, j, :], in_=xt_d[j, :, c0 : c0 + group]
                            )
                    for m in range(MT):
                        pys = [
                            py_pool.tile(
                                [128, BB], mybir.dt.float32, tag="py",
                                name=f"py_{bp}_{m}_{h}",
                            )
                            for h in range(group // BB)
                        ]
                        for j in range(KT):
                            w = w_sb[:, 0 if fixed_w else m * KT + j, :]
                            for h, py in enumerate(pys):
                                nc.tensor.matmul(
                                    py, w, xt[:, j, h * BB : (h + 1) * BB],
                                    start=(j == 0), stop=(j == KT - 1),
                                )
                        for h, py in enumerate(pys):
                            yo = yout_pool.tile([128, BB], IO_DT, tag="yo")
                            # bias-add + fp32->fp16 cast on copyback, DVE/ACT split
                            if (h + m) % 2 == 0:
                                nc.vector.tensor_scalar_add(
                                    yo, py, bias_sb[:, m : m + 1]
                                )
                            else:
                                nc.scalar.activation(
                                    yo, py,
                                    mybir.ActivationFunctionType.Identity,
                                    bias=bias_sb[:, m : m + 1],
                                )
                            out_eng = (
                                nc.scalar if out_ring == "act" else nc.sync
                            )
                            out_eng.dma_start(
                                out=yt_d[m, :, c0 + h * BB : c0 + (h + 1) * BB],
                                in_=yo,
                            )
    return nc


def _marshal(x):
    """x [B, C, NS] f32 -> per-core x^T [CT, BS] fp16 (contiguous)."""
    xf = np.asarray(x, np.float16).reshape(N_CORES, BS, CT)
    return np.ascontiguousarray(xf.transpose(0, 2, 1))  # [N_CORES, CT, BS]


def _run(x, Wflat, bias_pm, trace=False, tmpdir=None):
    nc = bacc.Bacc("TRN2", target_bir_lowering=False)
    build_kernel(nc, BS)
    nc.compile()
    xt = _marshal(x)
    wt = _w_tiles(Wflat)
    in_maps = [
        {"xt": xt[i].reshape(KT, 128, BS), "wt": wt, "bias_pm": bias_pm}
        for i in range(N_CORES)
    ]
    res = run_bass_kernel_spmd(
        nc, in_maps, list(range(N_CORES)), trace=trace, tmpdir=tmpdir
    )
    yt = np.stack(
        [res.results[i]["yt"].reshape(FS, BS) for i in range(N_CORES)]
    )  # [8, FS, BS]
    y = yt.transpose(0, 2, 1).astype(np.float32).reshape(B, F, NS)
    return y, res


def kernel(x, kernel, bias, irreps_d1, irreps_d2, irreps_d3):
    Wflat, bias_pm = _host_fold(kernel, bias, irreps_d1, irreps_d2, irreps_d3)
    y, _ = _run(np.asarray(x, dtype=np.float32), Wflat, bias_pm)
    return y
